# revision 46
# baseline (speedup 1.0000x reference)
"""Trainium2 Bass kernel for nn_Attention_83734682403408 (sliding-window sigmoid attention).

Sharding: 8 cores = (batch 2) x (sequence quarters 4). Each core processes 512
query tokens with a 64-token left halo for the W=64 local window.

Host prep per core: slice + zero-pad the halo'd x chunk, transpose x and the
weight matrices to contraction-major layout (free on host, same HBM bytes),
build bf16 rope tables for the chunk's absolute positions.

Per-core device pipeline (matmuls bf16 x bf16 -> fp32 PSUM):
  SWDGE cast-DMA fp32->bf16 directly into SBUF: xT [512,576], WqT, WkvT(k|v), WlinT
  Q^T  = WqT . xT           (feature-major) + rope         [512 f, 512 t]
  K-side tiles (feature-major, per head-pair), roped for heads 0-3
  V-side tiles (token-major,  per head-pair), roped for heads 0-3
  S^T[j,q] = K-chunk^T.T Q^T  per (pair, qtile, head)
  A = sigmoid(S/8 - log 64) * band-mask
  O^T[d,q] += V-chunk.T A^T   (col-tiled head concurrency)
  Y = O^T.T WlinT + ones.blin -> [512, 512] fp32 out
"""
import sys

if "/opt/trn_rl_repo" not in sys.path:
    sys.path.insert(0, "/opt/trn_rl_repo")

import math
import numpy as np
import ml_dtypes

B, T, QDIM = 2, 2048, 512
H, DH, W = 8, 64, 64
DM = H * DH
CHUNK = 512
HALO = 64
TH = HALO + CHUNK  # 576
NC = 8
LOG_W = math.log(W)
SCALE = DH ** -0.5

_cache = {}


def _host_tables(start):
    # match reference: fp32 inv_freq, fp32 t, fp32 angle
    inv_freq = (100.0 ** (-np.arange(0, QDIM, 2, dtype=np.float32) / QDIM)).astype(np.float32)
    t_q = np.arange(start, start + CHUNK, dtype=np.float32)
    ang_q = inv_freq[:, None] * t_q[None, :]
    t_k = np.arange(start - HALO, start + CHUNK, dtype=np.float32)
    fk = np.concatenate([np.arange(0, 64), np.arange(128, 192)])
    ang_k = inv_freq[fk][:, None] * t_k[None, :]
    fv = np.concatenate([np.arange(64, 128), np.arange(192, 256)])
    ang_v = t_k[:, None] * inv_freq[fv][None, :]
    bf = ml_dtypes.bfloat16
    return (np.cos(ang_q).astype(bf), np.sin(ang_q).astype(bf),
            np.cos(ang_k).astype(bf), np.sin(ang_k).astype(bf),
            np.cos(ang_v).astype(bf), np.sin(ang_v).astype(bf))


def _gen_nc():
    import concourse.bacc as bacc
    import concourse.mybir as mybir
    import concourse.tile as tile

    fp32 = mybir.dt.float32
    bf16 = mybir.dt.bfloat16
    AF = mybir.ActivationFunctionType
    ALU = mybir.AluOpType

    nc = bacc.Bacc(target_bir_lowering=False)

    # ------------- I/O (host passes contraction-major fp32 weights/x) -------------
    xt_d = nc.declare_dram_parameter("xhT", [QDIM, TH], fp32, isOutput=False)
    wqt_d = nc.declare_dram_parameter("WqT", [QDIM, DM], fp32, isOutput=False)
    wkt_d = nc.declare_dram_parameter("WkvTk", [QDIM, DM], fp32, isOutput=False)
    wvt_d = nc.declare_dram_parameter("WkvTv", [QDIM, DM], fp32, isOutput=False)
    wlt_d = nc.declare_dram_parameter("WlinT", [DM, DM], fp32, isOutput=False)
    bl_d = nc.declare_dram_parameter("blin", [DM], bf16, isOutput=False)
    cq_d = nc.declare_dram_parameter("cos_q", [256, CHUNK], bf16, isOutput=False)
    sq_d = nc.declare_dram_parameter("sin_q", [256, CHUNK], bf16, isOutput=False)
    ck_d = nc.declare_dram_parameter("cos_k", [128, TH], bf16, isOutput=False)
    sk_d = nc.declare_dram_parameter("sin_k", [128, TH], bf16, isOutput=False)
    cv_d = nc.declare_dram_parameter("cos_vt", [TH, 128], bf16, isOutput=False)
    sv_d = nc.declare_dram_parameter("sin_vt", [TH, 128], bf16, isOutput=False)
    y_d = nc.declare_dram_parameter("y", [CHUNK, DM], fp32, isOutput=True)

    with tile.TileContext(nc) as tc:
        with (
            tc.tile_pool(name="const", bufs=1) as cpool,
            tc.tile_pool(name="work", bufs=3) as wpool,
            tc.tile_pool(name="stage", bufs=1) as spool,
            tc.tile_pool(name="apool", bufs=16) as apool,
            tc.tile_pool(name="ps", bufs=8, space="PSUM") as pspool,
        ):
            def ctile(shape, dtype, nm):
                return cpool.tile(shape, dtype, name=nm, tag=nm)

            def pstile(nm):
                return pspool.tile([128, 512], fp32, name=nm, tag="ps")

            # ---------------- constants (fast Pool ops, before SWDGE gens) ----------------
            maskAB = ctile([128, 256], bf16, "maskAB")
            nc.gpsimd.memset(maskAB, 1.0)
            nc.gpsimd.affine_select(
                out=maskAB[:, 0:128], in_=maskAB[:, 0:128], compare_op=ALU.is_ge,
                fill=0.0, base=-1, pattern=[[-1, 128]], channel_multiplier=1)
            nc.gpsimd.affine_select(
                out=maskAB[:, 0:128], in_=maskAB[:, 0:128], compare_op=ALU.is_ge,
                fill=0.0, base=64, pattern=[[1, 128]], channel_multiplier=-1)
            nc.gpsimd.affine_select(
                out=maskAB[:, 128:256], in_=maskAB[:, 128:256], compare_op=ALU.is_ge,
                fill=0.0, base=-64, pattern=[[1, 128]], channel_multiplier=-1)
            maskW = ctile([128, 512], bf16, "maskW")
            nc.scalar.copy(maskW[:, 0:256], maskAB[:])
            nc.scalar.copy(maskW[:, 256:512], maskAB[:])
            ones = ctile([1, 128], bf16, "ones")
            nc.gpsimd.memset(ones, 1.0)
            ones_row = ctile([1, 512], bf16, "ones_row")
            nc.gpsimd.memset(ones_row, 1.0)
            sigb = ctile([128, 1], fp32, "sigb")  # sigmoid bias -log(W)
            nc.gpsimd.memset(sigb, -LOG_W)

            # PE warmup during the DMA front (inputs are memset-only consts)
            warm = pstile("warm")
            for _ in range(4):
                nc.tensor.matmul(warm[:], ones[:], ones_row[:], start=True, stop=True)

            # -------- cast-DMA weights / x straight into transposed SBUF tiles --------
            xT = ctile([128, 4, 640], bf16, "xT")
            wqT = ctile([128, 4, DM], bf16, "wqT")
            wkvT_k = ctile([128, 4, DM], bf16, "wkvT_k")
            wkvT_v = ctile([128, 4, DM], bf16, "wkvT_v")
            wlT = ctile([128, 4, DM], bf16, "wlT")
            nc.gpsimd.memset(xT[:, :, TH:640], 0.0)
            # critical-path tensors (x, Wkv_k) via HWDGE fp32 + DVE cast:
            # HWDGE first-byte ~0.6us vs ~2us SWDGE, and it frees the Pool
            # sequencer from descriptor generation.
            xT_f = spool.tile([128, 4, TH], fp32, name="xT_f", tag="xT_f")
            wk_f = spool.tile([128, 4, DM], fp32, name="wk_f", tag="wk_f")
            for h in range(2):
                rs = slice(h * 256, (h + 1) * 256)
                nc.sync.dma_start(xT_f[:, 2 * h:2 * h + 2, :],
                                  xt_d[rs, :].rearrange("(o p) t -> p o t", p=128))
                nc.sync.dma_start(wk_f[:, 2 * h:2 * h + 2, :],
                                  wkt_d[rs, :].rearrange("(o p) f -> p o f", p=128))
            for ko in range(4):
                if ko % 2 == 0:
                    nc.vector.tensor_copy(out=xT[:, ko, 0:TH], in_=xT_f[:, ko, :])
                    nc.scalar.copy(wkvT_k[:, ko, :], wk_f[:, ko, :])
                else:
                    nc.scalar.copy(xT[:, ko, 0:TH], xT_f[:, ko, :])
                    nc.vector.tensor_copy(out=wkvT_k[:, ko, :], in_=wk_f[:, ko, :])
            # remaining weights: SWDGE cast-DMA (off critical path)
            nc.gpsimd.dma_start(wqT[:],
                                wqt_d[:, :].rearrange("(o p) f -> p o f", p=128))
            nc.gpsimd.dma_start(wkvT_v[:],
                                wvt_d[:, :].rearrange("(o p) f -> p o f", p=128))
            nc.gpsimd.dma_start(wlT[:],
                                wlt_d[:, :].rearrange("(o p) f -> p o f", p=128))



            # ---------------- rope tables (bf16 inputs, HWDGE) ----------------
            ck = ctile([128, TH], bf16, "ck")
            sk = ctile([128, TH], bf16, "sk")
            nc.sync.dma_start(ck[:], ck_d[:, :])
            nc.sync.dma_start(sk[:], sk_d[:, :])
            cq = ctile([128, 2, CHUNK], bf16, "cq")
            sq = ctile([128, 2, CHUNK], bf16, "sq")
            nc.sync.dma_start(cq[:], cq_d[:, :].rearrange("(o p) t -> p o t", p=128))
            nc.sync.dma_start(sq[:], sq_d[:, :].rearrange("(o p) t -> p o t", p=128))
            cv = ctile([128, 5, 128], bf16, "cv")
            sv = ctile([128, 5, 128], bf16, "sv")
            nc.sync.dma_start(cv[:, 0:4, :], cv_d[0:512, :].rearrange("(o p) f -> p o f", p=128))
            nc.sync.dma_start(cv[0:64, 4, :], cv_d[512:TH, :])
            nc.sync.dma_start(sv[:, 0:4, :], sv_d[0:512, :].rearrange("(o p) f -> p o f", p=128))
            nc.sync.dma_start(sv[0:64, 4, :], sv_d[512:TH, :])
            blin = ctile([1, DM], bf16, "blin")
            nc.sync.dma_start(blin[:], bl_d[:].rearrange("(o d) -> o d", o=1))

            # ---------------- K-side feature-major projections ----------------
            kpk_raw0 = spool.tile([128, TH], bf16, name="kpk_raw0", tag="kpk_raw0")
            kpk_raw1 = spool.tile([128, TH], bf16, name="kpk_raw1", tag="kpk_raw1")
            kpk = ctile([128, 2, TH], bf16, "kpk")
            vpk = ctile([128, 2, TH], bf16, "vpk")
            for i in range(4):
                dst = [kpk_raw0[:], kpk_raw1[:], vpk[:, 0, :], vpk[:, 1, :]][i]
                ps1 = pstile("ps_k1")
                for ko in range(4):
                    nc.tensor.matmul(ps1[:], wkvT_k[:, ko, i * 128:(i + 1) * 128],
                                     xT[:, ko, 0:512],
                                     start=(ko == 0), stop=(ko == 3))
                ps2 = pstile("ps_k2")
                for ko in range(4):
                    nc.tensor.matmul(ps2[:, 0:64], wkvT_k[:, ko, i * 128:(i + 1) * 128],
                                     xT[:, ko, 512:TH],
                                     start=(ko == 0), stop=(ko == 3))
                nc.any.tensor_copy(out=dst[:, 0:512], in_=ps1[:])
                nc.any.tensor_copy(out=dst[:, 512:TH], in_=ps2[:, 0:64])
            # rope KPk (tile0 <-> tile1, freqs {0-63,128-191})
            tk1 = wpool.tile([128, TH], bf16, name="tk", tag="tk")
            nc.vector.tensor_tensor(tk1[:], kpk_raw1[:], sk[:], ALU.mult)
            nc.vector.tensor_tensor(kpk[:, 0, :], kpk_raw0[:], ck[:], ALU.mult)
            nc.vector.tensor_tensor(kpk[:, 0, :], kpk[:, 0, :], tk1[:], ALU.subtract)
            tk2 = wpool.tile([128, TH], bf16, name="tk", tag="tk")
            nc.vector.tensor_tensor(tk2[:], kpk_raw0[:], sk[:], ALU.mult)
            nc.vector.tensor_tensor(kpk[:, 1, :], kpk_raw1[:], ck[:], ALU.mult)
            nc.vector.tensor_tensor(kpk[:, 1, :], kpk[:, 1, :], tk2[:], ALU.add)

            # ---------------- Q^T projection + f-major rope ----------------
            qt_raw = spool.tile([128, 4, CHUNK], bf16, name="qt_raw", tag="qt_raw")
            for fo in range(4):
                ps = pstile("ps_q")
                for ko in range(4):
                    nc.tensor.matmul(ps[:], wqT[:, ko, fo * 128:(fo + 1) * 128],
                                     xT[:, ko, HALO:HALO + CHUNK],
                                     start=(ko == 0), stop=(ko == 3))
                nc.any.tensor_copy(out=qt_raw[:, fo, :], in_=ps[:])
            qt_r = ctile([128, 4, CHUNK], bf16, "qt_r")
            for pair, (a, b) in enumerate([(0, 2), (1, 3)]):
                c, s = cq[:, pair, :], sq[:, pair, :]
                t1 = wpool.tile([128, CHUNK], bf16, name="tmp", tag="tmp")
                nc.vector.tensor_tensor(t1[:], qt_raw[:, b, :], s, ALU.mult)
                nc.vector.tensor_tensor(qt_r[:, a, :], qt_raw[:, a, :], c, ALU.mult)
                nc.vector.tensor_tensor(qt_r[:, a, :], qt_r[:, a, :], t1[:], ALU.subtract)
                t2 = wpool.tile([128, CHUNK], bf16, name="tmp", tag="tmp")
                nc.vector.tensor_tensor(t2[:], qt_raw[:, a, :], s, ALU.mult)
                nc.vector.tensor_tensor(qt_r[:, b, :], qt_raw[:, b, :], c, ALU.mult)
                nc.vector.tensor_tensor(qt_r[:, b, :], qt_r[:, b, :], t2[:], ALU.add)

            # ---------------- attention ----------------
            # Emit all S^T matmuls first, then sigmoid/mask/AV per chain: the
            # PE stream stays dense (PSUM slot rotation forms the pipeline).
            ot = ctile([128, 4, CHUNK], bf16, "ot")
            groups = [(pi, qt) for pi in range(4) for qt in range(4)]
            psS_of = {}
            for pi, qt in groups:
                ktile = kpk[:, pi, :] if pi < 2 else vpk[:, pi - 2, :]
                j0 = qt * 128
                psS = pspool.tile([128, 512], fp32, name="psS", tag="ps")
                for hh in range(2):
                    hp, c0 = hh * 64, hh * 256
                    nc.tensor.matmul(psS[:, c0:c0 + 128],
                                     ktile[hp:hp + 64, j0:j0 + 128],
                                     qt_r[hp:hp + 64, pi, j0:j0 + 128],
                                     start=True, stop=True)
                    nc.tensor.matmul(psS[0:64, c0 + 128:c0 + 256],
                                     ktile[hp:hp + 64, j0 + 128:j0 + 192],
                                     qt_r[hp:hp + 64, pi, j0:j0 + 128],
                                     start=True, stop=True)
                psS_of[(pi, qt)] = psS

            # ---------------- V-side token-major projections ----------------
            kpv = ctile([128, 5, 256], bf16, "kpv")
            vpv = ctile([128, 5, 256], bf16, "vpv")
            for to in range(5):
                # cols 0:256 = KPv v-rows of kp (roped); 256:512 = VPv of vp
                ps = pstile("ps_v")
                for ko in range(4):
                    nc.tensor.matmul(ps[:], xT[:, ko, to * 128:(to + 1) * 128],
                                     wkvT_v[:, ko, 0:512],
                                     start=(ko == 0), stop=(ko == 3))
                kr = wpool.tile([128, 256], bf16, name="kvr", tag="kvr")
                nc.any.tensor_copy(out=kr[:], in_=ps[:, 0:256])
                tv1 = wpool.tile([128, 128], bf16, name="tv", tag="tv")
                nc.vector.tensor_tensor(tv1[:], kr[:, 128:256], sv[:, to, :], ALU.mult)
                nc.vector.tensor_tensor(kpv[:, to, 0:128], kr[:, 0:128], cv[:, to, :], ALU.mult)
                nc.vector.tensor_tensor(kpv[:, to, 0:128], kpv[:, to, 0:128], tv1[:], ALU.subtract)
                tv2 = wpool.tile([128, 128], bf16, name="tv", tag="tv")
                nc.vector.tensor_tensor(tv2[:], kr[:, 0:128], sv[:, to, :], ALU.mult)
                nc.vector.tensor_tensor(kpv[:, to, 128:256], kr[:, 128:256], cv[:, to, :], ALU.mult)
                nc.vector.tensor_tensor(kpv[:, to, 128:256], kpv[:, to, 128:256], tv2[:], ALU.add)
                # VPv (unroped): cols 256:512 of the same psum
                nc.any.tensor_copy(out=vpv[:, to, :], in_=ps[:, 256:512])

            psO_of = {}
            for pi, qt in groups:
                vtile = kpv if pi < 2 else vpv
                vcol = (pi % 2) * 128
                j0 = qt * 128
                psS = psS_of[(pi, qt)]
                a_sb = apool.tile([128, 512], bf16, name="a_sb", tag="a_sb")
                nc.scalar.activation(a_sb[:], psS[:], AF.Sigmoid,
                                     bias=sigb[:], scale=SCALE)
                nc.vector.tensor_tensor(a_sb[:], a_sb[:], maskW[:], ALU.mult)
                if qt == 0:
                    psO_of[pi] = pspool.tile([128, 512], fp32, name="psO", tag="ps")
                psO = psO_of[pi]
                for hh in range(2):
                    hp, c0 = hh * 64, hh * 256
                    nc.tensor.matmul(psO[hp:hp + 64, j0:j0 + 128],
                                     vtile[:, qt, vcol + hp:vcol + hp + 64],
                                     a_sb[:, c0:c0 + 128],
                                     start=True, stop=False, tile_position=(0, hp))
                    nc.tensor.matmul(psO[hp:hp + 64, j0:j0 + 128],
                                     vtile[0:64, qt + 1, vcol + hp:vcol + hp + 64],
                                     a_sb[0:64, c0 + 128:c0 + 256],
                                     start=False, stop=True, tile_position=(0, hp))
                if qt == 3:
                    nc.vector.tensor_copy(out=ot[:, pi, :], in_=psO[:])

            # ---------------- output projection ----------------
            y_sb = spool.tile([128, 4, DM], fp32, name="y_sb", tag="y_sb")
            for to in range(4):
                ps = pstile("ps_y")
                nc.tensor.matmul(ps[:], ones[:], blin[:], start=True, stop=False)
                for ko in range(4):
                    nc.tensor.matmul(ps[:], ot[:, ko, to * 128:(to + 1) * 128],
                                     wlT[:, ko, :],
                                     start=False, stop=(ko == 3))
                nc.any.tensor_copy(out=y_sb[:, to, :], in_=ps[:])
                nc.sync.dma_start(y_d[to * 128:(to + 1) * 128, :], y_sb[:, to, :])

    nc.finalize()
    return nc


def _get_nc():
    if "nc" not in _cache:
        _cache["nc"] = _gen_nc()
    return _cache["nc"]


def _make_in_maps(x, Wq, Wkv, Wlin, blin):
    f32 = np.float32
    wkv3 = Wkv.reshape(8, 128, QDIM)
    WkvTk = np.ascontiguousarray(wkv3[:, 0:64, :].reshape(512, QDIM).T, dtype=f32)
    WkvTv = np.ascontiguousarray(wkv3[:, 64:128, :].reshape(512, QDIM).T, dtype=f32)
    WqT = np.ascontiguousarray(Wq.T, dtype=f32)
    WlinT = np.ascontiguousarray(Wlin.T, dtype=f32)
    blin_bf = blin.astype(ml_dtypes.bfloat16)
    in_maps = []
    for core in range(NC):
        b, c = divmod(core, 4)
        start = c * CHUNK
        xh = np.zeros((TH, QDIM), f32)
        lo = max(0, start - HALO)
        xh[HALO - (start - lo):] = x[b, lo:start + CHUNK]
        xhT = np.ascontiguousarray(xh.T)
        cos_q, sin_q, cos_k, sin_k, cos_vt, sin_vt = _host_tables(start)
        in_maps.append({
            "xhT": xhT, "WqT": WqT, "WkvTk": WkvTk, "WkvTv": WkvTv,
            "WlinT": WlinT, "blin": blin_bf,
            "cos_q": cos_q, "sin_q": sin_q, "cos_k": cos_k, "sin_k": sin_k,
            "cos_vt": cos_vt, "sin_vt": sin_vt,
        })
    return in_maps


def _run(in_maps, **kw):
    from concourse.bass_utils import run_bass_kernel_spmd
    return run_bass_kernel_spmd(_get_nc(), in_maps, core_ids=list(range(NC)), **kw)


def kernel(x, mask, Wq, Wkv, Wlin, blin):
    x = np.ascontiguousarray(np.asarray(x, dtype=np.float32))
    Wq = np.ascontiguousarray(np.asarray(Wq, dtype=np.float32))
    Wkv = np.ascontiguousarray(np.asarray(Wkv, dtype=np.float32))
    Wlin = np.ascontiguousarray(np.asarray(Wlin, dtype=np.float32))
    blin = np.ascontiguousarray(np.asarray(blin, dtype=np.float32))

    res = _run(_make_in_maps(x, Wq, Wkv, Wlin, blin))
    out = np.zeros((B, T, DM), np.float32)
    for core in range(NC):
        b, c = divmod(core, 4)
        out[b, c * CHUNK:(c + 1) * CHUNK] = res.results[core]["y"]
    return out


# revision 51
# speedup vs baseline: 1.0340x; 1.0340x over previous
"""Trainium2 Bass kernel for nn_Attention_83734682403408 (sliding-window sigmoid attention).

Sharding: 8 cores = (batch 2) x (sequence quarters 4). Each core processes 512
query tokens with a 64-token left halo for the W=64 local window.

Host prep per core: slice + zero-pad the halo'd x chunk, transpose x and the
weight matrices to contraction-major layout (free on host, same HBM bytes),
build bf16 rope tables for the chunk's absolute positions.

Per-core device pipeline (matmuls bf16 x bf16 -> fp32 PSUM):
  SWDGE cast-DMA fp32->bf16 directly into SBUF: xT [512,576], WqT, WkvT(k|v), WlinT
  Q^T  = WqT . xT           (feature-major) + rope         [512 f, 512 t]
  K-side tiles (feature-major, per head-pair), roped for heads 0-3
  V-side tiles (token-major,  per head-pair), roped for heads 0-3
  S^T[j,q] = K-chunk^T.T Q^T  per (pair, qtile, head)
  A = sigmoid(S/8 - log 64) * band-mask
  O^T[d,q] += V-chunk.T A^T   (col-tiled head concurrency)
  Y = O^T.T WlinT + ones.blin -> [512, 512] fp32 out
"""
import sys

if "/opt/trn_rl_repo" not in sys.path:
    sys.path.insert(0, "/opt/trn_rl_repo")

import math
import numpy as np
import ml_dtypes

B, T, QDIM = 2, 2048, 512
H, DH, W = 8, 64, 64
DM = H * DH
CHUNK = 512
HALO = 64
TH = HALO + CHUNK  # 576
NC = 8
LOG_W = math.log(W)
SCALE = DH ** -0.5

_cache = {}


def _host_tables(start):
    # match reference: fp32 inv_freq, fp32 t, fp32 angle
    inv_freq = (100.0 ** (-np.arange(0, QDIM, 2, dtype=np.float32) / QDIM)).astype(np.float32)
    t_q = np.arange(start, start + CHUNK, dtype=np.float32)
    ang_q = inv_freq[:, None] * t_q[None, :]
    t_k = np.arange(start - HALO, start + CHUNK, dtype=np.float32)
    fk = np.concatenate([np.arange(0, 64), np.arange(128, 192)])
    ang_k = inv_freq[fk][:, None] * t_k[None, :]
    fv = np.concatenate([np.arange(64, 128), np.arange(192, 256)])
    ang_v = t_k[:, None] * inv_freq[fv][None, :]
    bf = ml_dtypes.bfloat16
    return (np.cos(ang_q).astype(bf), np.sin(ang_q).astype(bf),
            np.cos(ang_k).astype(bf), np.sin(ang_k).astype(bf),
            np.cos(ang_v).astype(bf), np.sin(ang_v).astype(bf))


def _gen_nc():
    import concourse.bacc as bacc
    import concourse.mybir as mybir
    import concourse.tile as tile

    fp32 = mybir.dt.float32
    bf16 = mybir.dt.bfloat16
    AF = mybir.ActivationFunctionType
    ALU = mybir.AluOpType

    nc = bacc.Bacc(target_bir_lowering=False)

    # ------------- I/O (host passes contraction-major fp32 weights/x) -------------
    xt_d = nc.declare_dram_parameter("xhT", [QDIM, TH], fp32, isOutput=False)
    wqt_d = nc.declare_dram_parameter("WqT", [QDIM, DM], fp32, isOutput=False)
    wkt_d = nc.declare_dram_parameter("WkvTk", [QDIM, DM], fp32, isOutput=False)
    wvt_d = nc.declare_dram_parameter("WkvTv", [QDIM, DM], fp32, isOutput=False)
    wlt_d = nc.declare_dram_parameter("WlinT", [DM, DM], fp32, isOutput=False)
    bl_d = nc.declare_dram_parameter("blin", [DM], bf16, isOutput=False)
    cq_d = nc.declare_dram_parameter("cos_q", [256, CHUNK], bf16, isOutput=False)
    sq_d = nc.declare_dram_parameter("sin_q", [256, CHUNK], bf16, isOutput=False)
    ck_d = nc.declare_dram_parameter("cos_k", [128, TH], bf16, isOutput=False)
    sk_d = nc.declare_dram_parameter("sin_k", [128, TH], bf16, isOutput=False)
    cv_d = nc.declare_dram_parameter("cos_vt", [TH, 128], bf16, isOutput=False)
    sv_d = nc.declare_dram_parameter("sin_vt", [TH, 128], bf16, isOutput=False)
    y_d = nc.declare_dram_parameter("y", [CHUNK, DM], fp32, isOutput=True)

    with tile.TileContext(nc) as tc:
        with (
            tc.tile_pool(name="const", bufs=1) as cpool,
            tc.tile_pool(name="work", bufs=3) as wpool,
            tc.tile_pool(name="stage", bufs=1) as spool,
            tc.tile_pool(name="apool", bufs=16) as apool,
            tc.tile_pool(name="ps", bufs=8, space="PSUM") as pspool,
        ):
            def ctile(shape, dtype, nm):
                return cpool.tile(shape, dtype, name=nm, tag=nm)

            def pstile(nm):
                return pspool.tile([128, 512], fp32, name=nm, tag="ps")

            # ---------------- constants (fast Pool ops, before SWDGE gens) ----------------
            sigb = ctile([128, 1], fp32, "sigb")  # sigmoid bias -log(W)
            nc.gpsimd.memset(sigb, -LOG_W)
            # dummy sigmoid as the FIRST ACT op: pins the "sigmoid_and_friends"
            # table set (which also serves Copy) so no mid-kernel table swap.
            sg_scr = wpool.tile([128, 1], bf16, name="sg_scr", tag="sg_scr")
            nc.scalar.activation(sg_scr[:], sigb[:], AF.Sigmoid)

            maskAB = ctile([128, 256], bf16, "maskAB")
            nc.gpsimd.memset(maskAB, 1.0)
            nc.gpsimd.affine_select(
                out=maskAB[:, 0:128], in_=maskAB[:, 0:128], compare_op=ALU.is_ge,
                fill=0.0, base=-1, pattern=[[-1, 128]], channel_multiplier=1)
            nc.gpsimd.affine_select(
                out=maskAB[:, 0:128], in_=maskAB[:, 0:128], compare_op=ALU.is_ge,
                fill=0.0, base=64, pattern=[[1, 128]], channel_multiplier=-1)
            nc.gpsimd.affine_select(
                out=maskAB[:, 128:256], in_=maskAB[:, 128:256], compare_op=ALU.is_ge,
                fill=0.0, base=-64, pattern=[[1, 128]], channel_multiplier=-1)
            maskW = ctile([128, 512], bf16, "maskW")
            nc.scalar.copy(maskW[:, 0:256], maskAB[:])
            nc.scalar.copy(maskW[:, 256:512], maskAB[:])
            ones = ctile([1, 128], bf16, "ones")
            nc.gpsimd.memset(ones, 1.0)
            ones_row = ctile([1, 512], bf16, "ones_row")
            nc.gpsimd.memset(ones_row, 1.0)

            # PE warmup during the DMA front (inputs are memset-only consts)
            warm = pstile("warm")
            for _ in range(4):
                nc.tensor.matmul(warm[:], ones[:], ones_row[:], start=True, stop=True)

            # -------- cast-DMA weights / x straight into transposed SBUF tiles --------
            xT = ctile([128, 4, 640], bf16, "xT")
            wqT = ctile([128, 4, DM], bf16, "wqT")
            wkvT_k = ctile([128, 4, DM], bf16, "wkvT_k")
            wkvT_v = ctile([128, 4, DM], bf16, "wkvT_v")
            wlT = ctile([128, 4, DM], bf16, "wlT")
            nc.gpsimd.memset(xT[:, :, TH:640], 0.0)
            # critical-path tensors (x, Wkv_k) via HWDGE fp32 + DVE cast:
            # HWDGE first-byte ~0.6us vs ~2us SWDGE, and it frees the Pool
            # sequencer from descriptor generation.
            xT_f = spool.tile([128, 4, TH], fp32, name="xT_f", tag="xT_f")
            wk_f = spool.tile([128, 4, DM], fp32, name="wk_f", tag="wk_f")
            for h in range(2):
                rs = slice(h * 256, (h + 1) * 256)
                nc.sync.dma_start(xT_f[:, 2 * h:2 * h + 2, :],
                                  xt_d[rs, :].rearrange("(o p) t -> p o t", p=128))
                nc.sync.dma_start(wk_f[:, 2 * h:2 * h + 2, :],
                                  wkt_d[rs, :].rearrange("(o p) f -> p o f", p=128))
            for ko in range(4):
                if ko % 2 == 0:
                    nc.vector.tensor_copy(out=xT[:, ko, 0:TH], in_=xT_f[:, ko, :])
                    nc.scalar.copy(wkvT_k[:, ko, :], wk_f[:, ko, :])
                else:
                    nc.scalar.copy(xT[:, ko, 0:TH], xT_f[:, ko, :])
                    nc.vector.tensor_copy(out=wkvT_k[:, ko, :], in_=wk_f[:, ko, :])
            # remaining weights: SWDGE cast-DMA (off critical path)
            nc.gpsimd.dma_start(wqT[:],
                                wqt_d[:, :].rearrange("(o p) f -> p o f", p=128))
            nc.gpsimd.dma_start(wkvT_v[:],
                                wvt_d[:, :].rearrange("(o p) f -> p o f", p=128))
            nc.gpsimd.dma_start(wlT[:],
                                wlt_d[:, :].rearrange("(o p) f -> p o f", p=128))



            # ---------------- rope tables (bf16 inputs, HWDGE) ----------------
            ck = ctile([128, TH], bf16, "ck")
            sk = ctile([128, TH], bf16, "sk")
            nc.sync.dma_start(ck[:], ck_d[:, :])
            nc.sync.dma_start(sk[:], sk_d[:, :])
            cq = ctile([128, 2, CHUNK], bf16, "cq")
            sq = ctile([128, 2, CHUNK], bf16, "sq")
            nc.sync.dma_start(cq[:], cq_d[:, :].rearrange("(o p) t -> p o t", p=128))
            nc.sync.dma_start(sq[:], sq_d[:, :].rearrange("(o p) t -> p o t", p=128))
            cv = ctile([128, 5, 128], bf16, "cv")
            sv = ctile([128, 5, 128], bf16, "sv")
            nc.sync.dma_start(cv[:, 0:4, :], cv_d[0:512, :].rearrange("(o p) f -> p o f", p=128))
            nc.sync.dma_start(cv[0:64, 4, :], cv_d[512:TH, :])
            nc.sync.dma_start(sv[:, 0:4, :], sv_d[0:512, :].rearrange("(o p) f -> p o f", p=128))
            nc.sync.dma_start(sv[0:64, 4, :], sv_d[512:TH, :])
            blin = ctile([1, DM], bf16, "blin")
            nc.sync.dma_start(blin[:], bl_d[:].rearrange("(o d) -> o d", o=1))

            # ---------------- K-side feature-major projections ----------------
            kpk_raw0 = spool.tile([128, TH], bf16, name="kpk_raw0", tag="kpk_raw0")
            kpk_raw1 = spool.tile([128, TH], bf16, name="kpk_raw1", tag="kpk_raw1")
            kpk = ctile([128, 2, TH], bf16, "kpk")
            vpk = ctile([128, 2, TH], bf16, "vpk")
            for i in range(4):
                dst = [kpk_raw0[:], kpk_raw1[:], vpk[:, 0, :], vpk[:, 1, :]][i]
                ps1 = pstile("ps_k1")
                for ko in range(4):
                    nc.tensor.matmul(ps1[:], wkvT_k[:, ko, i * 128:(i + 1) * 128],
                                     xT[:, ko, 0:512],
                                     start=(ko == 0), stop=(ko == 3))
                ps2 = pstile("ps_k2")
                for ko in range(4):
                    nc.tensor.matmul(ps2[:, 0:64], wkvT_k[:, ko, i * 128:(i + 1) * 128],
                                     xT[:, ko, 512:TH],
                                     start=(ko == 0), stop=(ko == 3))
                nc.any.tensor_copy(out=dst[:, 0:512], in_=ps1[:])
                nc.any.tensor_copy(out=dst[:, 512:TH], in_=ps2[:, 0:64])
            # rope KPk (tile0 <-> tile1, freqs {0-63,128-191})
            tk1 = wpool.tile([128, TH], bf16, name="tk", tag="tk")
            nc.vector.tensor_tensor(tk1[:], kpk_raw1[:], sk[:], ALU.mult)
            nc.vector.tensor_tensor(kpk[:, 0, :], kpk_raw0[:], ck[:], ALU.mult)
            nc.vector.tensor_tensor(kpk[:, 0, :], kpk[:, 0, :], tk1[:], ALU.subtract)
            tk2 = wpool.tile([128, TH], bf16, name="tk", tag="tk")
            nc.vector.tensor_tensor(tk2[:], kpk_raw0[:], sk[:], ALU.mult)
            nc.vector.tensor_tensor(kpk[:, 1, :], kpk_raw1[:], ck[:], ALU.mult)
            nc.vector.tensor_tensor(kpk[:, 1, :], kpk[:, 1, :], tk2[:], ALU.add)

            # ---------------- Q^T projection + f-major rope ----------------
            qt_raw = spool.tile([128, 4, CHUNK], bf16, name="qt_raw", tag="qt_raw")
            for fo in range(4):
                ps = pstile("ps_q")
                for ko in range(4):
                    nc.tensor.matmul(ps[:], wqT[:, ko, fo * 128:(fo + 1) * 128],
                                     xT[:, ko, HALO:HALO + CHUNK],
                                     start=(ko == 0), stop=(ko == 3))
                nc.any.tensor_copy(out=qt_raw[:, fo, :], in_=ps[:])
            qt_r = ctile([128, 4, CHUNK], bf16, "qt_r")
            for pair, (a, b) in enumerate([(0, 2), (1, 3)]):
                c, s = cq[:, pair, :], sq[:, pair, :]
                t1 = wpool.tile([128, CHUNK], bf16, name="tmp", tag="tmp")
                nc.vector.tensor_tensor(t1[:], qt_raw[:, b, :], s, ALU.mult)
                nc.vector.tensor_tensor(qt_r[:, a, :], qt_raw[:, a, :], c, ALU.mult)
                nc.vector.tensor_tensor(qt_r[:, a, :], qt_r[:, a, :], t1[:], ALU.subtract)
                t2 = wpool.tile([128, CHUNK], bf16, name="tmp", tag="tmp")
                nc.vector.tensor_tensor(t2[:], qt_raw[:, a, :], s, ALU.mult)
                nc.vector.tensor_tensor(qt_r[:, b, :], qt_raw[:, b, :], c, ALU.mult)
                nc.vector.tensor_tensor(qt_r[:, b, :], qt_r[:, b, :], t2[:], ALU.add)

            # ---------------- attention ----------------
            # Emit all S^T matmuls first, then sigmoid/mask/AV per chain: the
            # PE stream stays dense (PSUM slot rotation forms the pipeline).
            ot = ctile([128, 4, CHUNK], bf16, "ot")
            groups = [(pi, qt) for pi in range(4) for qt in range(4)]
            psS_of = {}
            for pi, qt in groups:
                ktile = kpk[:, pi, :] if pi < 2 else vpk[:, pi - 2, :]
                j0 = qt * 128
                psS = pspool.tile([128, 512], fp32, name="psS", tag="ps")
                for hh in range(2):
                    hp, c0 = hh * 64, hh * 256
                    nc.tensor.matmul(psS[:, c0:c0 + 128],
                                     ktile[hp:hp + 64, j0:j0 + 128],
                                     qt_r[hp:hp + 64, pi, j0:j0 + 128],
                                     start=True, stop=True)
                    nc.tensor.matmul(psS[0:64, c0 + 128:c0 + 256],
                                     ktile[hp:hp + 64, j0 + 128:j0 + 192],
                                     qt_r[hp:hp + 64, pi, j0:j0 + 128],
                                     start=True, stop=True)
                psS_of[(pi, qt)] = psS

            # ---------------- V-side token-major projections ----------------
            kpv = ctile([128, 5, 256], bf16, "kpv")
            vpv = ctile([128, 5, 256], bf16, "vpv")
            for to in range(5):
                # cols 0:256 = KPv v-rows of kp (roped); 256:512 = VPv of vp
                ps = pstile("ps_v")
                for ko in range(4):
                    nc.tensor.matmul(ps[:], xT[:, ko, to * 128:(to + 1) * 128],
                                     wkvT_v[:, ko, 0:512],
                                     start=(ko == 0), stop=(ko == 3))
                kr = wpool.tile([128, 256], bf16, name="kvr", tag="kvr")
                nc.any.tensor_copy(out=kr[:], in_=ps[:, 0:256])
                tv1 = wpool.tile([128, 128], bf16, name="tv", tag="tv")
                nc.vector.tensor_tensor(tv1[:], kr[:, 128:256], sv[:, to, :], ALU.mult)
                nc.vector.tensor_tensor(kpv[:, to, 0:128], kr[:, 0:128], cv[:, to, :], ALU.mult)
                nc.vector.tensor_tensor(kpv[:, to, 0:128], kpv[:, to, 0:128], tv1[:], ALU.subtract)
                tv2 = wpool.tile([128, 128], bf16, name="tv", tag="tv")
                nc.vector.tensor_tensor(tv2[:], kr[:, 0:128], sv[:, to, :], ALU.mult)
                nc.vector.tensor_tensor(kpv[:, to, 128:256], kr[:, 128:256], cv[:, to, :], ALU.mult)
                nc.vector.tensor_tensor(kpv[:, to, 128:256], kpv[:, to, 128:256], tv2[:], ALU.add)
                # VPv (unroped): cols 256:512 of the same psum
                nc.any.tensor_copy(out=vpv[:, to, :], in_=ps[:, 256:512])

            psO_of = {}
            for pi, qt in groups:
                vtile = kpv if pi < 2 else vpv
                vcol = (pi % 2) * 128
                j0 = qt * 128
                psS = psS_of[(pi, qt)]
                a_sb = apool.tile([128, 512], bf16, name="a_sb", tag="a_sb")
                nc.scalar.activation(a_sb[:], psS[:], AF.Sigmoid,
                                     bias=sigb[:], scale=SCALE)
                nc.vector.tensor_tensor(a_sb[:], a_sb[:], maskW[:], ALU.mult)
                if qt == 0:
                    psO_of[pi] = pspool.tile([128, 512], fp32, name="psO", tag="ps")
                psO = psO_of[pi]
                for hh in range(2):
                    hp, c0 = hh * 64, hh * 256
                    nc.tensor.matmul(psO[hp:hp + 64, j0:j0 + 128],
                                     vtile[:, qt, vcol + hp:vcol + hp + 64],
                                     a_sb[:, c0:c0 + 128],
                                     start=True, stop=False, tile_position=(0, hp))
                    nc.tensor.matmul(psO[hp:hp + 64, j0:j0 + 128],
                                     vtile[0:64, qt + 1, vcol + hp:vcol + hp + 64],
                                     a_sb[0:64, c0 + 128:c0 + 256],
                                     start=False, stop=True, tile_position=(0, hp))
                if qt == 3:
                    nc.vector.tensor_copy(out=ot[:, pi, :], in_=psO[:])

            # ---------------- output projection ----------------
            y_sb = spool.tile([128, 4, DM], fp32, name="y_sb", tag="y_sb")
            for to in range(4):
                ps = pstile("ps_y")
                nc.tensor.matmul(ps[:], ones[:], blin[:], start=True, stop=False)
                for ko in range(4):
                    nc.tensor.matmul(ps[:], ot[:, ko, to * 128:(to + 1) * 128],
                                     wlT[:, ko, :],
                                     start=False, stop=(ko == 3))
                nc.any.tensor_copy(out=y_sb[:, to, :], in_=ps[:])
                nc.sync.dma_start(y_d[to * 128:(to + 1) * 128, :], y_sb[:, to, :])

    nc.finalize()
    return nc


def _get_nc():
    if "nc" not in _cache:
        _cache["nc"] = _gen_nc()
    return _cache["nc"]


def _make_in_maps(x, Wq, Wkv, Wlin, blin):
    f32 = np.float32
    wkv3 = Wkv.reshape(8, 128, QDIM)
    WkvTk = np.ascontiguousarray(wkv3[:, 0:64, :].reshape(512, QDIM).T, dtype=f32)
    WkvTv = np.ascontiguousarray(wkv3[:, 64:128, :].reshape(512, QDIM).T, dtype=f32)
    WqT = np.ascontiguousarray(Wq.T, dtype=f32)
    WlinT = np.ascontiguousarray(Wlin.T, dtype=f32)
    blin_bf = blin.astype(ml_dtypes.bfloat16)
    in_maps = []
    for core in range(NC):
        b, c = divmod(core, 4)
        start = c * CHUNK
        xh = np.zeros((TH, QDIM), f32)
        lo = max(0, start - HALO)
        xh[HALO - (start - lo):] = x[b, lo:start + CHUNK]
        xhT = np.ascontiguousarray(xh.T)
        cos_q, sin_q, cos_k, sin_k, cos_vt, sin_vt = _host_tables(start)
        in_maps.append({
            "xhT": xhT, "WqT": WqT, "WkvTk": WkvTk, "WkvTv": WkvTv,
            "WlinT": WlinT, "blin": blin_bf,
            "cos_q": cos_q, "sin_q": sin_q, "cos_k": cos_k, "sin_k": sin_k,
            "cos_vt": cos_vt, "sin_vt": sin_vt,
        })
    return in_maps


def _run(in_maps, **kw):
    from concourse.bass_utils import run_bass_kernel_spmd
    return run_bass_kernel_spmd(_get_nc(), in_maps, core_ids=list(range(NC)), **kw)


def kernel(x, mask, Wq, Wkv, Wlin, blin):
    x = np.ascontiguousarray(np.asarray(x, dtype=np.float32))
    Wq = np.ascontiguousarray(np.asarray(Wq, dtype=np.float32))
    Wkv = np.ascontiguousarray(np.asarray(Wkv, dtype=np.float32))
    Wlin = np.ascontiguousarray(np.asarray(Wlin, dtype=np.float32))
    blin = np.ascontiguousarray(np.asarray(blin, dtype=np.float32))

    res = _run(_make_in_maps(x, Wq, Wkv, Wlin, blin))
    out = np.zeros((B, T, DM), np.float32)
    for core in range(NC):
        b, c = divmod(core, 4)
        out[b, c * CHUNK:(c + 1) * CHUNK] = res.results[core]["y"]
    return out


# revision 58
# speedup vs baseline: 1.0343x; 1.0003x over previous
"""Trainium2 Bass kernel for nn_Attention_83734682403408 (sliding-window sigmoid attention).

Sharding: 8 cores = (batch 2) x (sequence quarters 4). Each core processes 512
query tokens with a 64-token left halo for the W=64 local window.

Host prep per core: slice + zero-pad the halo'd x chunk, transpose x and the
weight matrices to contraction-major layout (free on host, same HBM bytes),
build bf16 rope tables for the chunk's absolute positions.

Per-core device pipeline (matmuls bf16 x bf16 -> fp32 PSUM):
  SWDGE cast-DMA fp32->bf16 directly into SBUF: xT [512,576], WqT, WkvT(k|v), WlinT
  Q^T  = WqT . xT           (feature-major) + rope         [512 f, 512 t]
  K-side tiles (feature-major, per head-pair), roped for heads 0-3
  V-side tiles (token-major,  per head-pair), roped for heads 0-3
  S^T[j,q] = K-chunk^T.T Q^T  per (pair, qtile, head)
  A = sigmoid(S/8 - log 64) * band-mask
  O^T[d,q] += V-chunk.T A^T   (col-tiled head concurrency)
  Y = O^T.T WlinT + ones.blin -> [512, 512] fp32 out
"""
import sys

if "/opt/trn_rl_repo" not in sys.path:
    sys.path.insert(0, "/opt/trn_rl_repo")

import math
import numpy as np
import ml_dtypes

B, T, QDIM = 2, 2048, 512
H, DH, W = 8, 64, 64
DM = H * DH
CHUNK = 512
HALO = 64
TH = HALO + CHUNK  # 576
NC = 8
LOG_W = math.log(W)
SCALE = DH ** -0.5

_cache = {}


def _host_tables(start):
    # match reference: fp32 inv_freq, fp32 t, fp32 angle
    inv_freq = (100.0 ** (-np.arange(0, QDIM, 2, dtype=np.float32) / QDIM)).astype(np.float32)
    t_q = np.arange(start, start + CHUNK, dtype=np.float32)
    ang_q = inv_freq[:, None] * t_q[None, :]
    t_k = np.arange(start - HALO, start + CHUNK, dtype=np.float32)
    fk = np.concatenate([np.arange(0, 64), np.arange(128, 192)])
    ang_k = inv_freq[fk][:, None] * t_k[None, :]
    fv = np.concatenate([np.arange(64, 128), np.arange(192, 256)])
    ang_v = t_k[:, None] * inv_freq[fv][None, :]
    bf = ml_dtypes.bfloat16
    return (np.cos(ang_q).astype(bf), np.sin(ang_q).astype(bf),
            np.cos(ang_k).astype(bf), np.sin(ang_k).astype(bf),
            np.cos(ang_v).astype(bf), np.sin(ang_v).astype(bf))


def _gen_nc():
    import concourse.bacc as bacc
    import concourse.mybir as mybir
    import concourse.tile as tile

    fp32 = mybir.dt.float32
    bf16 = mybir.dt.bfloat16
    AF = mybir.ActivationFunctionType
    ALU = mybir.AluOpType

    nc = bacc.Bacc(target_bir_lowering=False)

    # ------------- I/O (host passes contraction-major fp32 weights/x) -------------
    xt_d = nc.declare_dram_parameter("xhT", [QDIM, TH], fp32, isOutput=False)
    wqt_d = nc.declare_dram_parameter("WqT", [QDIM, DM], fp32, isOutput=False)
    wkt_d = nc.declare_dram_parameter("WkvTk", [QDIM, DM], fp32, isOutput=False)
    wvt_d = nc.declare_dram_parameter("WkvTv", [QDIM, DM], fp32, isOutput=False)
    wlt_d = nc.declare_dram_parameter("WlinT", [DM, DM], fp32, isOutput=False)
    bl_d = nc.declare_dram_parameter("blin", [DM], bf16, isOutput=False)
    cq_d = nc.declare_dram_parameter("cos_q", [256, CHUNK], bf16, isOutput=False)
    sq_d = nc.declare_dram_parameter("sin_q", [256, CHUNK], bf16, isOutput=False)
    ck_d = nc.declare_dram_parameter("cos_k", [128, TH], bf16, isOutput=False)
    sk_d = nc.declare_dram_parameter("sin_k", [128, TH], bf16, isOutput=False)
    cv_d = nc.declare_dram_parameter("cos_vt", [TH, 128], bf16, isOutput=False)
    sv_d = nc.declare_dram_parameter("sin_vt", [TH, 128], bf16, isOutput=False)
    y_d = nc.declare_dram_parameter("y", [CHUNK, DM], fp32, isOutput=True)

    with tile.TileContext(nc) as tc:
        with (
            tc.tile_pool(name="const", bufs=1) as cpool,
            tc.tile_pool(name="work", bufs=3) as wpool,
            tc.tile_pool(name="stage", bufs=1) as spool,
            tc.tile_pool(name="apool", bufs=16) as apool,
            tc.tile_pool(name="ps", bufs=8, space="PSUM") as pspool,
        ):
            def ctile(shape, dtype, nm):
                return cpool.tile(shape, dtype, name=nm, tag=nm)

            def pstile(nm):
                return pspool.tile([128, 512], fp32, name=nm, tag="ps")

            # ---------------- constants (fast Pool ops, before SWDGE gens) ----------------
            sigb = ctile([128, 1], fp32, "sigb")  # sigmoid bias -log(W)
            nc.gpsimd.memset(sigb, -LOG_W)
            # dummy sigmoid as the FIRST ACT op: pins the "sigmoid_and_friends"
            # table set (which also serves Copy) so no mid-kernel table swap.
            sg_scr = wpool.tile([128, 1], bf16, name="sg_scr", tag="sg_scr")
            nc.scalar.activation(sg_scr[:], sigb[:], AF.Sigmoid)

            maskAB = ctile([128, 256], bf16, "maskAB")
            nc.gpsimd.memset(maskAB, 1.0)
            nc.gpsimd.affine_select(
                out=maskAB[:, 0:128], in_=maskAB[:, 0:128], compare_op=ALU.is_ge,
                fill=0.0, base=-1, pattern=[[-1, 128]], channel_multiplier=1)
            nc.gpsimd.affine_select(
                out=maskAB[:, 0:128], in_=maskAB[:, 0:128], compare_op=ALU.is_ge,
                fill=0.0, base=64, pattern=[[1, 128]], channel_multiplier=-1)
            nc.gpsimd.affine_select(
                out=maskAB[:, 128:256], in_=maskAB[:, 128:256], compare_op=ALU.is_ge,
                fill=0.0, base=-64, pattern=[[1, 128]], channel_multiplier=-1)
            ones = ctile([1, 128], bf16, "ones")
            nc.gpsimd.memset(ones, 1.0)
            ones_row = ctile([1, 512], bf16, "ones_row")
            nc.gpsimd.memset(ones_row, 1.0)

            # PE warmup during the DMA front (inputs are memset-only consts)
            warm = pstile("warm")
            for _ in range(4):
                nc.tensor.matmul(warm[:], ones[:], ones_row[:], start=True, stop=True)

            # -------- cast-DMA weights / x straight into transposed SBUF tiles --------
            xT = ctile([128, 4, 640], bf16, "xT")
            wqT = ctile([128, 4, DM], bf16, "wqT")
            wkvT_k = ctile([128, 4, DM], bf16, "wkvT_k")
            wkvT_v = ctile([128, 4, DM], bf16, "wkvT_v")
            wlT = ctile([128, 4, DM], bf16, "wlT")
            nc.gpsimd.memset(xT[:, :, TH:640], 0.0)
            # critical-path tensors (x, Wkv_k) via HWDGE fp32 + DVE cast:
            # HWDGE first-byte ~0.6us vs ~2us SWDGE, and it frees the Pool
            # sequencer from descriptor generation.
            xT_f = spool.tile([128, 4, TH], fp32, name="xT_f", tag="xT_f")
            wk_f = spool.tile([128, 4, DM], fp32, name="wk_f", tag="wk_f")
            for h in range(2):
                rs = slice(h * 256, (h + 1) * 256)
                nc.sync.dma_start(xT_f[:, 2 * h:2 * h + 2, :],
                                  xt_d[rs, :].rearrange("(o p) t -> p o t", p=128))
                nc.scalar.dma_start(wk_f[:, 2 * h:2 * h + 2, :],
                                    wkt_d[rs, :].rearrange("(o p) f -> p o f", p=128))
            for ko in range(4):
                if ko % 2 == 0:
                    nc.vector.tensor_copy(out=xT[:, ko, 0:TH], in_=xT_f[:, ko, :])
                    nc.scalar.copy(wkvT_k[:, ko, :], wk_f[:, ko, :])
                else:
                    nc.scalar.copy(xT[:, ko, 0:TH], xT_f[:, ko, :])
                    nc.vector.tensor_copy(out=wkvT_k[:, ko, :], in_=wk_f[:, ko, :])
            # remaining weights: SWDGE cast-DMA (off critical path)
            nc.gpsimd.dma_start(wqT[:],
                                wqt_d[:, :].rearrange("(o p) f -> p o f", p=128))
            nc.gpsimd.dma_start(wkvT_v[:],
                                wvt_d[:, :].rearrange("(o p) f -> p o f", p=128))
            nc.gpsimd.dma_start(wlT[:],
                                wlt_d[:, :].rearrange("(o p) f -> p o f", p=128))



            maskW = ctile([128, 512], bf16, "maskW")
            nc.scalar.copy(maskW[:, 0:256], maskAB[:])
            nc.scalar.copy(maskW[:, 256:512], maskAB[:])

            # ---------------- rope tables (bf16 inputs, HWDGE) ----------------
            ck = ctile([128, TH], bf16, "ck")
            sk = ctile([128, TH], bf16, "sk")
            nc.sync.dma_start(ck[:], ck_d[:, :])
            nc.sync.dma_start(sk[:], sk_d[:, :])
            cq = ctile([128, 2, CHUNK], bf16, "cq")
            sq = ctile([128, 2, CHUNK], bf16, "sq")
            nc.sync.dma_start(cq[:], cq_d[:, :].rearrange("(o p) t -> p o t", p=128))
            nc.sync.dma_start(sq[:], sq_d[:, :].rearrange("(o p) t -> p o t", p=128))
            cv = ctile([128, 5, 128], bf16, "cv")
            sv = ctile([128, 5, 128], bf16, "sv")
            nc.sync.dma_start(cv[:, 0:4, :], cv_d[0:512, :].rearrange("(o p) f -> p o f", p=128))
            nc.sync.dma_start(cv[0:64, 4, :], cv_d[512:TH, :])
            nc.sync.dma_start(sv[:, 0:4, :], sv_d[0:512, :].rearrange("(o p) f -> p o f", p=128))
            nc.sync.dma_start(sv[0:64, 4, :], sv_d[512:TH, :])
            blin = ctile([1, DM], bf16, "blin")
            nc.sync.dma_start(blin[:], bl_d[:].rearrange("(o d) -> o d", o=1))

            # ---------------- K-side feature-major projections ----------------
            kpk_raw0 = spool.tile([128, TH], bf16, name="kpk_raw0", tag="kpk_raw0")
            kpk_raw1 = spool.tile([128, TH], bf16, name="kpk_raw1", tag="kpk_raw1")
            kpk = ctile([128, 2, TH], bf16, "kpk")
            vpk = ctile([128, 2, TH], bf16, "vpk")
            for i in range(4):
                dst = [kpk_raw0[:], kpk_raw1[:], vpk[:, 0, :], vpk[:, 1, :]][i]
                ps1 = pstile("ps_k1")
                for ko in range(4):
                    nc.tensor.matmul(ps1[:], wkvT_k[:, ko, i * 128:(i + 1) * 128],
                                     xT[:, ko, 0:512],
                                     start=(ko == 0), stop=(ko == 3))
                ps2 = pstile("ps_k2")
                for ko in range(4):
                    nc.tensor.matmul(ps2[:, 0:64], wkvT_k[:, ko, i * 128:(i + 1) * 128],
                                     xT[:, ko, 512:TH],
                                     start=(ko == 0), stop=(ko == 3))
                nc.any.tensor_copy(out=dst[:, 0:512], in_=ps1[:])
                nc.any.tensor_copy(out=dst[:, 512:TH], in_=ps2[:, 0:64])
            # rope KPk (tile0 <-> tile1, freqs {0-63,128-191})
            tk1 = wpool.tile([128, TH], bf16, name="tk", tag="tk")
            nc.vector.tensor_tensor(tk1[:], kpk_raw1[:], sk[:], ALU.mult)
            nc.vector.tensor_tensor(kpk[:, 0, :], kpk_raw0[:], ck[:], ALU.mult)
            nc.vector.tensor_tensor(kpk[:, 0, :], kpk[:, 0, :], tk1[:], ALU.subtract)
            tk2 = wpool.tile([128, TH], bf16, name="tk", tag="tk")
            nc.vector.tensor_tensor(tk2[:], kpk_raw0[:], sk[:], ALU.mult)
            nc.vector.tensor_tensor(kpk[:, 1, :], kpk_raw1[:], ck[:], ALU.mult)
            nc.vector.tensor_tensor(kpk[:, 1, :], kpk[:, 1, :], tk2[:], ALU.add)

            # ---------------- Q^T projection + f-major rope ----------------
            qt_raw = spool.tile([128, 4, CHUNK], bf16, name="qt_raw", tag="qt_raw")
            for fo in range(4):
                ps = pstile("ps_q")
                for ko in range(4):
                    nc.tensor.matmul(ps[:], wqT[:, ko, fo * 128:(fo + 1) * 128],
                                     xT[:, ko, HALO:HALO + CHUNK],
                                     start=(ko == 0), stop=(ko == 3))
                nc.any.tensor_copy(out=qt_raw[:, fo, :], in_=ps[:])
            qt_r = ctile([128, 4, CHUNK], bf16, "qt_r")
            for pair, (a, b) in enumerate([(0, 2), (1, 3)]):
                c, s = cq[:, pair, :], sq[:, pair, :]
                t1 = wpool.tile([128, CHUNK], bf16, name="tmp", tag="tmp")
                nc.vector.tensor_tensor(t1[:], qt_raw[:, b, :], s, ALU.mult)
                nc.vector.tensor_tensor(qt_r[:, a, :], qt_raw[:, a, :], c, ALU.mult)
                nc.vector.tensor_tensor(qt_r[:, a, :], qt_r[:, a, :], t1[:], ALU.subtract)
                t2 = wpool.tile([128, CHUNK], bf16, name="tmp", tag="tmp")
                nc.vector.tensor_tensor(t2[:], qt_raw[:, a, :], s, ALU.mult)
                nc.vector.tensor_tensor(qt_r[:, b, :], qt_raw[:, b, :], c, ALU.mult)
                nc.vector.tensor_tensor(qt_r[:, b, :], qt_r[:, b, :], t2[:], ALU.add)

            # ---------------- attention ----------------
            # Emit all S^T matmuls first, then sigmoid/mask/AV per chain: the
            # PE stream stays dense (PSUM slot rotation forms the pipeline).
            ot = ctile([128, 4, CHUNK], bf16, "ot")
            groups = [(pi, qt) for pi in range(4) for qt in range(4)]
            psS_of = {}
            for pi, qt in groups:
                ktile = kpk[:, pi, :] if pi < 2 else vpk[:, pi - 2, :]
                j0 = qt * 128
                psS = pspool.tile([128, 512], fp32, name="psS", tag="ps")
                for hh in range(2):
                    hp, c0 = hh * 64, hh * 256
                    nc.tensor.matmul(psS[:, c0:c0 + 128],
                                     ktile[hp:hp + 64, j0:j0 + 128],
                                     qt_r[hp:hp + 64, pi, j0:j0 + 128],
                                     start=True, stop=True)
                    nc.tensor.matmul(psS[0:64, c0 + 128:c0 + 256],
                                     ktile[hp:hp + 64, j0 + 128:j0 + 192],
                                     qt_r[hp:hp + 64, pi, j0:j0 + 128],
                                     start=True, stop=True)
                psS_of[(pi, qt)] = psS

            # ---------------- V-side token-major projections ----------------
            kpv = ctile([128, 5, 256], bf16, "kpv")
            vpv = ctile([128, 5, 256], bf16, "vpv")
            for to in range(5):
                # cols 0:256 = KPv v-rows of kp (roped); 256:512 = VPv of vp
                ps = pstile("ps_v")
                for ko in range(4):
                    nc.tensor.matmul(ps[:], xT[:, ko, to * 128:(to + 1) * 128],
                                     wkvT_v[:, ko, 0:512],
                                     start=(ko == 0), stop=(ko == 3))
                kr = wpool.tile([128, 256], bf16, name="kvr", tag="kvr")
                nc.any.tensor_copy(out=kr[:], in_=ps[:, 0:256])
                tv1 = wpool.tile([128, 128], bf16, name="tv", tag="tv")
                nc.vector.tensor_tensor(tv1[:], kr[:, 128:256], sv[:, to, :], ALU.mult)
                nc.vector.tensor_tensor(kpv[:, to, 0:128], kr[:, 0:128], cv[:, to, :], ALU.mult)
                nc.vector.tensor_tensor(kpv[:, to, 0:128], kpv[:, to, 0:128], tv1[:], ALU.subtract)
                tv2 = wpool.tile([128, 128], bf16, name="tv", tag="tv")
                nc.vector.tensor_tensor(tv2[:], kr[:, 0:128], sv[:, to, :], ALU.mult)
                nc.vector.tensor_tensor(kpv[:, to, 128:256], kr[:, 128:256], cv[:, to, :], ALU.mult)
                nc.vector.tensor_tensor(kpv[:, to, 128:256], kpv[:, to, 128:256], tv2[:], ALU.add)
                # VPv (unroped): cols 256:512 of the same psum
                nc.any.tensor_copy(out=vpv[:, to, :], in_=ps[:, 256:512])

            psO_of = {}
            for pi, qt in groups:
                vtile = kpv if pi < 2 else vpv
                vcol = (pi % 2) * 128
                j0 = qt * 128
                psS = psS_of[(pi, qt)]
                a_sb = apool.tile([128, 512], bf16, name="a_sb", tag="a_sb")
                nc.scalar.activation(a_sb[:], psS[:], AF.Sigmoid,
                                     bias=sigb[:], scale=SCALE)
                nc.vector.tensor_tensor(a_sb[:], a_sb[:], maskW[:], ALU.mult)
                if qt == 0:
                    psO_of[pi] = pspool.tile([128, 512], fp32, name="psO", tag="ps")
                psO = psO_of[pi]
                for hh in range(2):
                    hp, c0 = hh * 64, hh * 256
                    nc.tensor.matmul(psO[hp:hp + 64, j0:j0 + 128],
                                     vtile[:, qt, vcol + hp:vcol + hp + 64],
                                     a_sb[:, c0:c0 + 128],
                                     start=True, stop=False, tile_position=(0, hp))
                    nc.tensor.matmul(psO[hp:hp + 64, j0:j0 + 128],
                                     vtile[0:64, qt + 1, vcol + hp:vcol + hp + 64],
                                     a_sb[0:64, c0 + 128:c0 + 256],
                                     start=False, stop=True, tile_position=(0, hp))
                if qt == 3:
                    nc.vector.tensor_copy(out=ot[:, pi, :], in_=psO[:])

            # ---------------- output projection ----------------
            y_sb = spool.tile([128, 4, DM], fp32, name="y_sb", tag="y_sb")
            for to in range(4):
                ps = pstile("ps_y")
                nc.tensor.matmul(ps[:], ones[:], blin[:], start=True, stop=False)
                for ko in range(4):
                    nc.tensor.matmul(ps[:], ot[:, ko, to * 128:(to + 1) * 128],
                                     wlT[:, ko, :],
                                     start=False, stop=(ko == 3))
                if to % 2 == 0:
                    nc.vector.tensor_copy(out=y_sb[:, to, :], in_=ps[:])
                else:
                    nc.scalar.copy(y_sb[:, to, :], ps[:])
                nc.sync.dma_start(y_d[to * 128:(to + 1) * 128, :], y_sb[:, to, :])

    nc.finalize()
    return nc


def _get_nc():
    if "nc" not in _cache:
        _cache["nc"] = _gen_nc()
    return _cache["nc"]


def _make_in_maps(x, Wq, Wkv, Wlin, blin):
    f32 = np.float32
    wkv3 = Wkv.reshape(8, 128, QDIM)
    WkvTk = np.ascontiguousarray(wkv3[:, 0:64, :].reshape(512, QDIM).T, dtype=f32)
    WkvTv = np.ascontiguousarray(wkv3[:, 64:128, :].reshape(512, QDIM).T, dtype=f32)
    WqT = np.ascontiguousarray(Wq.T, dtype=f32)
    WlinT = np.ascontiguousarray(Wlin.T, dtype=f32)
    blin_bf = blin.astype(ml_dtypes.bfloat16)
    in_maps = []
    for core in range(NC):
        b, c = divmod(core, 4)
        start = c * CHUNK
        xh = np.zeros((TH, QDIM), f32)
        lo = max(0, start - HALO)
        xh[HALO - (start - lo):] = x[b, lo:start + CHUNK]
        xhT = np.ascontiguousarray(xh.T)
        cos_q, sin_q, cos_k, sin_k, cos_vt, sin_vt = _host_tables(start)
        in_maps.append({
            "xhT": xhT, "WqT": WqT, "WkvTk": WkvTk, "WkvTv": WkvTv,
            "WlinT": WlinT, "blin": blin_bf,
            "cos_q": cos_q, "sin_q": sin_q, "cos_k": cos_k, "sin_k": sin_k,
            "cos_vt": cos_vt, "sin_vt": sin_vt,
        })
    return in_maps


def _run(in_maps, **kw):
    from concourse.bass_utils import run_bass_kernel_spmd
    return run_bass_kernel_spmd(_get_nc(), in_maps, core_ids=list(range(NC)), **kw)


def kernel(x, mask, Wq, Wkv, Wlin, blin):
    x = np.ascontiguousarray(np.asarray(x, dtype=np.float32))
    Wq = np.ascontiguousarray(np.asarray(Wq, dtype=np.float32))
    Wkv = np.ascontiguousarray(np.asarray(Wkv, dtype=np.float32))
    Wlin = np.ascontiguousarray(np.asarray(Wlin, dtype=np.float32))
    blin = np.ascontiguousarray(np.asarray(blin, dtype=np.float32))

    res = _run(_make_in_maps(x, Wq, Wkv, Wlin, blin))
    out = np.zeros((B, T, DM), np.float32)
    for core in range(NC):
        b, c = divmod(core, 4)
        out[b, c * CHUNK:(c + 1) * CHUNK] = res.results[core]["y"]
    return out
